# revision 13
# baseline (speedup 1.0000x reference)
"""Trainium2 Bass kernel: cross-modal channel attention (flipped-layout bf16,
software-pipelined across batches).

Math (per batch b), with G the static [L, S] linear-interp matrix:
    qT   = img_feat[b]^T                          [S, C]   (host pre-transposed, bf16)
    tp   = text_feat[b] @ W_txt                   [L, C]
    GQT  = G @ qT                                 [L, C]
    logits^T = tp^T @ GQT * S^-0.5                [Cj, Ci]
    E^T  = exp(logits^T)                          [Cj, Ci]
    ZTa  = tpTa^T @ E^T                           [80, Ci]  (row 0 = Z via ones col)
    ZT'  = (gamma * ZTa) * bcast(1/Z)             [80, Ci]
    out^T= qT + G_aug^T @ ZT'                     [S, C]    (residual via DVE adds)

Three-stage software pipeline over the 4 per-core batches so the PE never
stalls on the Act/DVE conversions between matmul phases:
    iter i: front(i)=tp+GQT mms | mid(i-1)=trans+logits+exp+ZT | back(i-2)=outA+resid
Sharding: data-parallel over batch across 8 cores; consts replicated.
Host does layout only: transpose/cast img->qT, text->txt^T, out^T->out.
"""

import sys

sys.path.insert(0, "/opt/trn_rl_repo")

from contextlib import ExitStack

import ml_dtypes
import numpy as np

import concourse.bacc as bacc
import concourse.mybir as mybir
import concourse.tile as tile
from concourse.bass_utils import run_bass_kernel_spmd
from concourse.masks import make_identity

B, C, HH, WW = 32, 768, 32, 32
S = HH * WW
L, D = 77, 512
N_CORES = 8
B_CORE = B // N_CORES
P = 128
CT, ST, DT = C // P, S // P, D // P
F32 = mybir.dt.float32
BF16 = mybir.dt.bfloat16
SCALE = float(S) ** -0.5
EXP = mybir.ActivationFunctionType.Exp
MULT = mybir.AluOpType.mult
ADD = mybir.AluOpType.add
NPBF = ml_dtypes.bfloat16

# Augmented-L layout: col/row 0 = ones/Z, 1 = zero, 2..78 = l 0..76, 79 = zero.
LA = 80


def _interp_matrix():
    """G[l, s] such that (tp^T @ G)[c, s] == linear_interp(tp^T, S)[c, s]."""
    src = np.clip(
        (np.arange(S, dtype=np.float32) + np.float32(0.5)) * np.float32(L / S)
        - np.float32(0.5),
        np.float32(0.0),
        np.float32(L - 1),
    )
    i0 = np.floor(src).astype(np.int32)
    i1 = np.minimum(i0 + 1, L - 1)
    w = (src - i0.astype(np.float32)).astype(np.float32)
    g = np.zeros((L, S), dtype=np.float32)
    g[i0, np.arange(S)] += np.float32(1.0) - w
    g[i1, np.arange(S)] += w
    return g


def _build():
    nc = bacc.Bacc("TRN2", target_bir_lowering=False, debug=False)
    img = nc.dram_tensor("imgT", [B_CORE, S, C], BF16, kind="ExternalInput").ap()
    txt = nc.dram_tensor("txtT", [B_CORE, D, L], BF16, kind="ExternalInput").ap()
    wt = nc.dram_tensor("wt", [D, C], BF16, kind="ExternalInput").ap()
    g = nc.dram_tensor("ga", [LA, S], BF16, kind="ExternalInput").ap()
    gt = nc.dram_tensor("gt", [S, L], BF16, kind="ExternalInput").ap()
    gamma = nc.dram_tensor("gammacol", [P, 1], F32, kind="ExternalInput").ap()
    out = nc.dram_tensor("outT", [B_CORE, S, C], BF16, kind="ExternalOutput").ap()

    with ExitStack() as ctx:
        ctx.enter_context(
            nc.allow_low_precision(reason="bf16 I/O fits the 2e-2 rel-err budget")
        )
        tc = ctx.enter_context(tile.TileContext(nc))
        consts = ctx.enter_context(tc.tile_pool(name="consts", bufs=1))
        q_pool = ctx.enter_context(tc.tile_pool(name="q", bufs=5))
        txt_pool = ctx.enter_context(tc.tile_pool(name="txtp", bufs=3))
        sb2 = ctx.enter_context(tc.tile_pool(name="sb2", bufs=2))
        et_pool = ctx.enter_context(tc.tile_pool(name="et", bufs=2))
        outp = ctx.enter_context(tc.tile_pool(name="outp", bufs=2))
        # PSUM: med tag (tp/gqt/trans/zt) 2x2 banks + big tag (logits/outA)
        # 2x2 banks = 8 banks total.
        ps_med = ctx.enter_context(tc.tile_pool(name="ps_m", bufs=2, space="PSUM"))
        ps_big = ctx.enter_context(tc.tile_pool(name="ps_b", bufs=2, space="PSUM"))

        qT = [None] * B_CORE
        txts = [None] * B_CORE
        tp_sb = [None] * B_CORE
        gqt_sb = [None] * B_CORE
        et_sb = [None] * B_CORE
        tpa_sb = [None] * B_CORE
        zt_sb = [None] * B_CORE
        gz_sb = [None] * B_CORE
        gzb_sb = [None] * B_CORE
        ztp_sb = [None] * B_CORE

        def dma_txt(b):
            txts[b] = txt_pool.tile([P, DT, L], BF16, tag="txt", name=f"txts{b}")
            nc.sync.dma_start(txts[b][:], txt[b].rearrange("(k p) l -> p k l", p=P))

        def dma_q(b):
            qT[b] = q_pool.tile([P, ST, C], BF16, tag="q", name=f"qT{b}")
            nc.sync.dma_start(qT[b][:], img[b].rearrange("(st p) c -> p st c", p=P))

        def dmas(b):
            dma_q(b)
            dma_txt(b)

        # DMA issue order = transfer order (single DMA lane): feed the first
        # compute phases first, bulky consts later.
        dma_txt(0)
        w_sb = consts.tile([P, DT, C], BF16)
        nc.sync.dma_start(w_sb[:], wt.rearrange("(k p) c -> p k c", p=P))
        dma_q(0)
        gt_sb = consts.tile([P, ST, L], BF16)
        nc.sync.dma_start(gt_sb[:], gt.rearrange("(st p) l -> p st l", p=P))
        dma_txt(1)
        dma_q(1)
        g_sb = consts.tile([P, S], BF16)
        nc.sync.dma_start(g_sb[0:LA, :], g)
        gamma_sb = consts.tile([P, 1], F32)
        nc.sync.dma_start(gamma_sb[:], gamma)
        ident = consts.tile([P, P], F32)
        make_identity(nc, ident[:])
        ident_bf = consts.tile([P, P], BF16)
        nc.vector.tensor_copy(ident_bf[:], ident[:])

        def front(b):
            # tp = text @ W_txt [L, C]
            ps_tp = ps_med.tile([P, C], F32, tag="med")
            for half, (c0, c1) in enumerate(((0, 512), (512, 768))):
                for k in range(DT):
                    nc.tensor.matmul(
                        ps_tp[0:L, c0:c1],
                        txts[b][:, k, :],
                        w_sb[:, k, c0:c1],
                        start=(k == 0),
                        stop=(k == DT - 1),
                    )
            tp_sb[b] = sb2.tile([P, C], BF16, tag="tp", name=f"tp{b}")
            nc.vector.tensor_copy(tp_sb[b][0:L, :], ps_tp[0:L, :])
            # GQT = G @ qT [L, C]
            ps_gqt = ps_med.tile([P, C], F32, tag="med")
            for c0, c1 in ((0, 512), (512, 768)):
                for st in range(ST):
                    nc.tensor.matmul(
                        ps_gqt[0:L, c0:c1],
                        gt_sb[:, st, :],
                        qT[b][:, st, c0:c1],
                        start=(st == 0),
                        stop=(st == ST - 1),
                    )
            gqt_sb[b] = sb2.tile([P, C], BF16, tag="gqt", name=f"gqt{b}")
            nc.vector.tensor_copy(gqt_sb[b][0:L, :], ps_gqt[0:L, :])

        def mid_head(b):
            # tp^T (augmented): col 0 ones, col 1 zero, cols 2:80 = tp^T + zero pad
            ps_tr = ps_med.tile([P, CT, LA], BF16, tag="med")
            for jt in range(CT):
                nc.tensor.transpose(
                    ps_tr[:, jt, 2:80],
                    tp_sb[b][0:L, jt * P : (jt + 1) * P],
                    ident_bf[0:L, 0:78],
                )
            tpa_sb[b] = sb2.tile([P, CT, LA], BF16, tag="tpa", name=f"tpa{b}")
            nc.gpsimd.memset(tpa_sb[b][:, :, 0:1], 1.0)
            nc.gpsimd.memset(tpa_sb[b][:, :, 1:2], 0.0)
            nc.vector.tensor_copy(tpa_sb[b][:, :, 2:80], ps_tr[:, :, 2:80])
            et_sb[b] = et_pool.tile([P, CT, C], BF16, tag="et", name=f"et{b}")

        def logits_pair(b, jt):
            # logits^T for one j-tile, fused exp -> E^T (bf16)
            psl = ps_big.tile([P, C], F32, tag="big")
            lhsT = tp_sb[b][0:L, jt * P : (jt + 1) * P]
            for c0, c1 in ((0, 512), (512, 768)):
                nc.tensor.matmul(
                    psl[:, c0:c1], lhsT, gqt_sb[b][0:L, c0:c1], start=True, stop=True
                )
            nc.scalar.activation(et_sb[b][:, jt, :], psl[:, 0:C], EXP, scale=SCALE)

        def mid_tail(b):
            # ZTa = tpTa^T @ E^T [LA, C]; row 0 = Z
            ps_zt = ps_med.tile([P, C], F32, tag="med")
            for c0, c1 in ((0, 512), (512, 768)):
                for jt in range(CT):
                    nc.tensor.matmul(
                        ps_zt[0:LA, c0:c1],
                        tpa_sb[b][:, jt, :],
                        et_sb[b][:, jt, c0:c1],
                        start=(jt == 0),
                        stop=(jt == CT - 1),
                    )
            # 1/Z row; gamma*ZTa (gamma folded into the PSUM->SBUF conv scale)
            gz_sb[b] = sb2.tile([P, C], BF16, tag="gz", name=f"gz{b}")
            nc.vector.reciprocal(gz_sb[b][0:1, :], ps_zt[0:1, :])
            zt_sb[b] = sb2.tile([P, C], BF16, tag="zt", name=f"zt{b}")
            nc.scalar.activation(
                zt_sb[b][0:LA, :],
                ps_zt[0:LA, :],
                mybir.ActivationFunctionType.Copy,
                scale=gamma_sb[0:LA, :],
            )
            gzb_sb[b] = sb2.tile([P, C], BF16, tag="gzb", name=f"gzb{b}")
            nc.gpsimd.partition_broadcast(gzb_sb[b][0:LA, :], gz_sb[b][0:1, :])

        def backpre(b):
            # ZT' = (gamma*ZTa) * bcast(1/Z)   (all-SBUF bf16 -> DVE 2x eligible)
            ztp_sb[b] = sb2.tile([P, C], BF16, tag="ztp", name=f"ztp{b}")
            nc.vector.tensor_tensor(
                ztp_sb[b][0:LA, :], zt_sb[b][0:LA, :], gzb_sb[b][0:LA, :], op=MULT
            )

        ACT_ST = (1, 4, 6)  # these s-tiles add the residual on PE, convert on Act

        def out_group(b, st, out_sb):
            pso = ps_big.tile([P, C], F32, tag="big")
            lhsT = g_sb[0:LA, st * P : (st + 1) * P]
            on_act = st in ACT_ST
            for c0, c1 in ((0, 512), (512, 768)):
                nc.tensor.matmul(
                    pso[:, c0:c1],
                    lhsT,
                    ztp_sb[b][0:LA, c0:c1],
                    start=True,
                    stop=not on_act,
                )
                if on_act:
                    nc.tensor.matmul(
                        pso[:, c0:c1],
                        ident_bf[:],
                        qT[b][:, st, c0:c1],
                        start=False,
                        stop=True,
                    )
            if on_act:
                nc.scalar.copy(out_sb[:, st, :], pso[:, 0:C])
            else:
                nc.vector.tensor_tensor(
                    out_sb[:, st, :], pso[:, 0:C], qT[b][:, st, :], op=ADD
                )

        def midback(bm, bb):
            # Interleave outA groups (batch bb) between logits pairs (batch bm)
            # so the PE fills the exp-wait and residual-wait bubbles.
            if bm is not None:
                mid_head(bm)
            out_sb = None
            if bb is not None:
                out_sb = outp.tile([P, ST, C], BF16, tag="o")
            for k in range(ST):
                if bb is not None:
                    out_group(bb, k, out_sb)
                if bm is not None and k < CT:
                    logits_pair(bm, k)
            if bm is not None:
                mid_tail(bm)
            if bb is not None:
                nc.sync.dma_start(
                    out[bb].rearrange("(st p) c -> p st c", p=P), out_sb[:]
                )

        for i in range(B_CORE + 2):
            if i >= 2:
                backpre(i - 2)
            if i + 2 < B_CORE:
                dmas(i + 2)
            if i < B_CORE:
                front(i)
            midback(
                i - 1 if 1 <= i <= B_CORE else None,
                i - 2 if i >= 2 else None,
            )

    nc.compile()
    return nc


_NC = None


def _get_nc():
    global _NC
    if _NC is None:
        _NC = _build()
    return _NC


def _in_maps(img_feat, text_feat, W_txt, gamma):
    imgT = (
        np.ascontiguousarray(img_feat, dtype=np.float32)
        .reshape(B, C, S)
        .transpose(0, 2, 1)
        .astype(NPBF)
    )
    txtT = (
        np.ascontiguousarray(text_feat, dtype=np.float32)
        .transpose(0, 2, 1)
        .astype(NPBF)
    )
    wt = np.ascontiguousarray(W_txt, dtype=np.float32).astype(NPBF)
    g = _interp_matrix()
    ga = np.zeros((LA, S), dtype=np.float32)
    ga[2 : 2 + L] = g
    ga = ga.astype(NPBF)
    gt = np.ascontiguousarray(g.T).astype(NPBF)
    gammacol = np.full((P, 1), np.float32(gamma.reshape(-1)[0]), dtype=np.float32)
    maps = []
    for m in range(N_CORES):
        sl = slice(m * B_CORE, (m + 1) * B_CORE)
        maps.append(
            {
                "imgT": np.ascontiguousarray(imgT[sl]),
                "txtT": np.ascontiguousarray(txtT[sl]),
                "wt": wt,
                "ga": ga,
                "gt": gt,
                "gammacol": gammacol,
            }
        )
    return maps


def _run(in_maps, **kwargs):
    nc = _get_nc()
    return run_bass_kernel_spmd(nc, in_maps, core_ids=list(range(N_CORES)), **kwargs)


def kernel(img_feat, text_feat, W_txt, gamma):
    res = _run(_in_maps(img_feat, text_feat, W_txt, gamma))
    full = np.concatenate(
        [np.asarray(res.results[m]["outT"]) for m in range(N_CORES)], axis=0
    )
    full = full.astype(np.float32).transpose(0, 2, 1)
    return np.ascontiguousarray(full).reshape(B, C, HH, WW)


# revision 17
# speedup vs baseline: 1.1191x; 1.1191x over previous
"""Trainium2 Bass kernel: cross-modal channel attention (flipped-layout bf16,
software-pipelined across batches).

Math (per batch b), with G the static [L, S] linear-interp matrix:
    qT   = img_feat[b]^T                          [S, C]   (host pre-transposed, bf16)
    tp   = text_feat[b] @ W_txt                   [L, C]
    GQT  = G @ qT                                 [L, C]
    logits^T = tp^T @ GQT * S^-0.5                [Cj, Ci]
    E^T  = exp(logits^T)                          [Cj, Ci]
    ZTa  = tpTa^T @ E^T                           [80, Ci]  (row 0 = Z via ones col)
    ZT'  = (gamma * ZTa) * bcast(1/Z)             [80, Ci]
    out^T= qT + G_aug^T @ ZT'                     [S, C]    (residual via DVE adds)

Three-stage software pipeline over the 4 per-core batches so the PE never
stalls on the Act/DVE conversions between matmul phases:
    iter i: front(i)=tp+GQT mms | mid(i-1)=trans+logits+exp+ZT | back(i-2)=outA+resid
Sharding: data-parallel over batch across 8 cores; consts replicated.
Host does layout only: transpose/cast img->qT, text->txt^T, out^T->out.
"""

import sys

sys.path.insert(0, "/opt/trn_rl_repo")

from contextlib import ExitStack

import ml_dtypes
import numpy as np

import concourse.bacc as bacc
import concourse.mybir as mybir
import concourse.tile as tile
from concourse.bass_utils import run_bass_kernel_spmd
from concourse.masks import make_identity

B, C, HH, WW = 32, 768, 32, 32
S = HH * WW
L, D = 77, 512
N_CORES = 8
B_CORE = B // N_CORES
P = 128
CT, ST, DT = C // P, S // P, D // P
F32 = mybir.dt.float32
BF16 = mybir.dt.bfloat16
SCALE = float(S) ** -0.5
EXP = mybir.ActivationFunctionType.Exp
MULT = mybir.AluOpType.mult
ADD = mybir.AluOpType.add
NPBF = ml_dtypes.bfloat16
F8 = mybir.dt.float8e4
NPF8 = ml_dtypes.float8_e4m3
DR = mybir.MatmulPerfMode.DoubleRow

# Augmented-L layout: col/row 0 = ones/Z, 1 = zero, 2..78 = l 0..76, 79 = zero.
LA = 80


def _interp_matrix():
    """G[l, s] such that (tp^T @ G)[c, s] == linear_interp(tp^T, S)[c, s]."""
    src = np.clip(
        (np.arange(S, dtype=np.float32) + np.float32(0.5)) * np.float32(L / S)
        - np.float32(0.5),
        np.float32(0.0),
        np.float32(L - 1),
    )
    i0 = np.floor(src).astype(np.int32)
    i1 = np.minimum(i0 + 1, L - 1)
    w = (src - i0.astype(np.float32)).astype(np.float32)
    g = np.zeros((L, S), dtype=np.float32)
    g[i0, np.arange(S)] += np.float32(1.0) - w
    g[i1, np.arange(S)] += w
    return g


def _build():
    nc = bacc.Bacc("TRN2", target_bir_lowering=False, debug=False)
    img = nc.dram_tensor("imgT", [B_CORE, S, C], BF16, kind="ExternalInput").ap()
    txt = nc.dram_tensor("txtT", [B_CORE, D, L], F8, kind="ExternalInput").ap()
    wt = nc.dram_tensor("wt", [D, C], F8, kind="ExternalInput").ap()
    ones8 = nc.dram_tensor("ones8", [P, CT, 1], F8, kind="ExternalInput").ap()
    g = nc.dram_tensor("ga", [LA, S], BF16, kind="ExternalInput").ap()
    gt = nc.dram_tensor("gt", [S, L], BF16, kind="ExternalInput").ap()
    gamma = nc.dram_tensor("gammacol", [P, 1], F32, kind="ExternalInput").ap()
    out = nc.dram_tensor("outT", [B_CORE, S, C], BF16, kind="ExternalOutput").ap()

    with ExitStack() as ctx:
        ctx.enter_context(
            nc.allow_low_precision(reason="bf16 I/O fits the 2e-2 rel-err budget")
        )
        tc = ctx.enter_context(tile.TileContext(nc))
        consts = ctx.enter_context(tc.tile_pool(name="consts", bufs=1))
        q_pool = ctx.enter_context(tc.tile_pool(name="q", bufs=5))
        txt_pool = ctx.enter_context(tc.tile_pool(name="txtp", bufs=3))
        sb2 = ctx.enter_context(tc.tile_pool(name="sb2", bufs=2))
        et_pool = ctx.enter_context(tc.tile_pool(name="et", bufs=2))
        outp = ctx.enter_context(tc.tile_pool(name="outp", bufs=2))
        # PSUM: med tag (tp/gqt/trans/zt) 2x2 banks + big tag (logits/outA)
        # 2x2 banks = 8 banks total.
        ps_med = ctx.enter_context(tc.tile_pool(name="ps_m", bufs=2, space="PSUM"))
        ps_big = ctx.enter_context(tc.tile_pool(name="ps_b", bufs=2, space="PSUM"))

        qT = [None] * B_CORE
        txts = [None] * B_CORE
        tp_sb = [None] * B_CORE
        gqt_sb = [None] * B_CORE
        et_sb = [None] * B_CORE
        tpa_sb = [None] * B_CORE
        zt_sb = [None] * B_CORE
        gz_sb = [None] * B_CORE
        gzb_sb = [None] * B_CORE
        ztp_sb = [None] * B_CORE

        def dma_txt(b):
            txts[b] = txt_pool.tile([P, DT, L], F8, tag="txt", name=f"txts{b}")
            nc.sync.dma_start(txts[b][:], txt[b].rearrange("(k p) l -> p k l", p=P))

        def dma_q(b):
            qT[b] = q_pool.tile([P, ST, C], BF16, tag="q", name=f"qT{b}")
            nc.sync.dma_start(qT[b][:], img[b].rearrange("(st p) c -> p st c", p=P))

        def dmas(b):
            dma_q(b)
            dma_txt(b)

        # DMA issue order = transfer order (single DMA lane): feed the first
        # compute phases first, bulky consts later.
        dma_txt(0)
        w_sb = consts.tile([P, DT, C], F8)
        nc.sync.dma_start(w_sb[:], wt.rearrange("(k p) c -> p k c", p=P))
        dma_q(0)
        gt_sb = consts.tile([P, ST, L], BF16)
        nc.sync.dma_start(gt_sb[:], gt.rearrange("(st p) l -> p st l", p=P))
        dma_txt(1)
        dma_q(1)
        g_sb = consts.tile([P, S], BF16)
        nc.sync.dma_start(g_sb[0:LA, :], g)
        gamma_sb = consts.tile([P, 1], F32)
        nc.sync.dma_start(gamma_sb[:], gamma)
        ident = consts.tile([P, P], F32)
        make_identity(nc, ident[:])
        ident_bf = consts.tile([P, P], BF16)
        nc.vector.tensor_copy(ident_bf[:], ident[:])
        ident_f8 = consts.tile([P, P], F8)
        nc.vector.tensor_copy(ident_f8[:], ident[:])
        ones8_sb = consts.tile([P, CT, 1], F8)
        nc.sync.dma_start(ones8_sb[:], ones8)
        neg2 = consts.tile([P, 1], F32)
        nc.gpsimd.memset(neg2[:], -2.0)

        def front(b):
            # tp = text @ W_txt [L, C]
            ps_tp = ps_med.tile([P, C], F32, tag="med")
            for c0, c1 in ((0, 512), (512, 768)):
                for k in range(DT):
                    nc.tensor.matmul(
                        ps_tp[0:L, c0:c1],
                        txts[b][:, k, :],
                        w_sb[:, k, c0:c1],
                        start=(k == 0),
                        stop=(k == DT - 1),
                    )
            tp_sb[b] = sb2.tile([P, C], BF16, tag="tp", name=f"tp{b}")
            nc.vector.tensor_copy(tp_sb[b][0:L, :], ps_tp[0:L, :])
            # GQT = G @ qT [L, C]
            ps_gqt = ps_med.tile([P, C], F32, tag="med")
            for c0, c1 in ((0, 512), (512, 768)):
                for st in range(ST):
                    nc.tensor.matmul(
                        ps_gqt[0:L, c0:c1],
                        gt_sb[:, st, :],
                        qT[b][:, st, c0:c1],
                        start=(st == 0),
                        stop=(st == ST - 1),
                    )
            gqt_sb[b] = sb2.tile([P, C], BF16, tag="gqt", name=f"gqt{b}")
            nc.vector.tensor_copy(gqt_sb[b][0:L, :], ps_gqt[0:L, :])

        def mid_head(b):
            # tp^T (augmented): col 0 ones, col 1 zero, cols 2:80 = tp^T + zero pad
            ps_tr = ps_med.tile([P, CT, LA], BF16, tag="med")
            for jt in range(CT):
                nc.tensor.transpose(
                    ps_tr[:, jt, 2:80],
                    tp_sb[b][0:L, jt * P : (jt + 1) * P],
                    ident_bf[0:L, 0:78],
                )
            tpa_sb[b] = sb2.tile([P, CT, LA], BF16, tag="tpa", name=f"tpa{b}")
            nc.gpsimd.memset(tpa_sb[b][:, :, 0:1], 1.0)
            nc.gpsimd.memset(tpa_sb[b][:, :, 1:2], 0.0)
            nc.vector.tensor_copy(tpa_sb[b][:, :, 2:80], ps_tr[:, :, 2:80])
            et_sb[b] = et_pool.tile([P, CT, C], BF16, tag="et", name=f"et{b}")

        def logits_pair(b, jt):
            # logits^T for one j-tile, fused exp -> E^T (bf16)
            psl = ps_big.tile([P, C], F32, tag="big")
            lhsT = tp_sb[b][0:L, jt * P : (jt + 1) * P]
            for c0, c1 in ((0, 512), (512, 768)):
                nc.tensor.matmul(
                    psl[:, c0:c1], lhsT, gqt_sb[b][0:L, c0:c1], start=True, stop=True
                )
            nc.scalar.activation(
                et_sb[b][:, jt, :], psl[:, 0:C], EXP, scale=SCALE
            )

        def mid_tail(b):
            # ZTa = tpTa^T @ E^T [LA, C]; row 0 = Z
            ps_zt = ps_med.tile([P, C], F32, tag="med")
            for c0, c1 in ((0, 512), (512, 768)):
                for jt in range(CT):
                    nc.tensor.matmul(
                        ps_zt[0:LA, c0:c1],
                        tpa_sb[b][:, jt, :],
                        et_sb[b][:, jt, c0:c1],
                        start=(jt == 0),
                        stop=(jt == CT - 1),
                    )
            # 1/Z row; gamma*ZTa (gamma folded into the PSUM->SBUF conv scale)
            gz_sb[b] = sb2.tile([P, C], BF16, tag="gz", name=f"gz{b}")
            nc.vector.reciprocal(gz_sb[b][0:1, :], ps_zt[0:1, :])
            zt_sb[b] = sb2.tile([P, C], BF16, tag="zt", name=f"zt{b}")
            nc.scalar.activation(
                zt_sb[b][0:LA, :],
                ps_zt[0:LA, :],
                mybir.ActivationFunctionType.Copy,
                scale=gamma_sb[0:LA, :],
            )
            gzb_sb[b] = sb2.tile([P, C], BF16, tag="gzb", name=f"gzb{b}")
            nc.gpsimd.partition_broadcast(gzb_sb[b][0:LA, :], gz_sb[b][0:1, :])

        def backpre(b):
            # ZT' = (gamma*ZTa) * bcast(1/Z)   (all-SBUF bf16 -> DVE 2x eligible)
            ztp_sb[b] = sb2.tile([P, C], BF16, tag="ztp", name=f"ztp{b}")
            nc.vector.tensor_tensor(
                ztp_sb[b][0:LA, :], zt_sb[b][0:LA, :], gzb_sb[b][0:LA, :], op=MULT
            )

        ACT_ST = (1, 4, 6)  # these s-tiles add the residual on PE, convert on Act

        def out_group(b, st, out_sb):
            pso = ps_big.tile([P, C], F32, tag="big")
            lhsT = g_sb[0:LA, st * P : (st + 1) * P]
            on_act = st in ACT_ST
            for c0, c1 in ((0, 512), (512, 768)):
                nc.tensor.matmul(
                    pso[:, c0:c1],
                    lhsT,
                    ztp_sb[b][0:LA, c0:c1],
                    start=True,
                    stop=not on_act,
                )
                if on_act:
                    nc.tensor.matmul(
                        pso[:, c0:c1],
                        ident_bf[:],
                        qT[b][:, st, c0:c1],
                        start=False,
                        stop=True,
                    )
            if on_act:
                nc.scalar.copy(out_sb[:, st, :], pso[:, 0:C])
            else:
                nc.vector.tensor_tensor(
                    out_sb[:, st, :], pso[:, 0:C], qT[b][:, st, :], op=ADD
                )

        def midback(bm, bb):
            # Interleave outA groups (batch bb) between logits pairs (batch bm)
            # so the PE fills the exp-wait and residual-wait bubbles.
            if bm is not None:
                mid_head(bm)
            if bm is not None:
                for jt in range(CT):
                    logits_pair(bm, jt)
                mid_tail(bm)
            out_sb = None
            if bb is not None:
                out_sb = outp.tile([P, ST, C], BF16, tag="o")
                for st in range(ST):
                    out_group(bb, st, out_sb)
            if bb is not None:
                nc.sync.dma_start(
                    out[bb].rearrange("(st p) c -> p st c", p=P), out_sb[:]
                )

        for i in range(B_CORE + 2):
            if i >= 2:
                backpre(i - 2)
            if i + 2 < B_CORE:
                dmas(i + 2)
            if i < B_CORE:
                front(i)
            midback(
                i - 1 if 1 <= i <= B_CORE else None,
                i - 2 if i >= 2 else None,
            )

    nc.compile()
    return nc


_NC = None


def _get_nc():
    global _NC
    if _NC is None:
        _NC = _build()
    return _NC


def _in_maps(img_feat, text_feat, W_txt, gamma):
    imgT = (
        np.ascontiguousarray(img_feat, dtype=np.float32)
        .reshape(B, C, S)
        .transpose(0, 2, 1)
        .astype(NPBF)
    )
    txtT = (
        np.ascontiguousarray(text_feat, dtype=np.float32)
        .transpose(0, 2, 1)
        .astype(NPF8)
    )
    wt = np.ascontiguousarray(W_txt, dtype=np.float32).astype(NPF8)
    ones8np = np.ones((P, CT, 1), dtype=np.float32).astype(NPF8)
    g = _interp_matrix()
    ga = np.zeros((LA, S), dtype=np.float32)
    ga[2 : 2 + L] = g
    ga = ga.astype(NPBF)
    gt = np.ascontiguousarray(g.T).astype(NPBF)
    gammacol = np.full((P, 1), np.float32(gamma.reshape(-1)[0]), dtype=np.float32)
    maps = []
    for m in range(N_CORES):
        sl = slice(m * B_CORE, (m + 1) * B_CORE)
        maps.append(
            {
                "imgT": np.ascontiguousarray(imgT[sl]),
                "txtT": np.ascontiguousarray(txtT[sl]),
                "wt": wt,
                "ga": ga,
                "gt": gt,
                "gammacol": gammacol,
                "ones8": ones8np,
            }
        )
    return maps


def _run(in_maps, **kwargs):
    nc = _get_nc()
    return run_bass_kernel_spmd(nc, in_maps, core_ids=list(range(N_CORES)), **kwargs)


def kernel(img_feat, text_feat, W_txt, gamma):
    res = _run(_in_maps(img_feat, text_feat, W_txt, gamma))
    full = np.concatenate(
        [np.asarray(res.results[m]["outT"]) for m in range(N_CORES)], axis=0
    )
    full = full.astype(np.float32).transpose(0, 2, 1)
    return np.ascontiguousarray(full).reshape(B, C, HH, WW)
